# revision 3
# baseline (speedup 1.0000x reference)
"""GNN message-passing (scatter-add) kernel for 8 Trainium2 NeuronCores.

Computes out = segment_sum(x[src], dst, num_segments=N) for
x [10000, 128] f32, edge_index [2, 320000] int64.

Strategy — BIT-PLANE dense count-matrix matmul (no gathers for the bulk):
  out^T[f, d] = sum_s A[s, d] * x[s, f]   with A[s, d] = #edges s->d.

  A is ~0/1 (density 0.3%, counts almost never exceed 1). Store
  min(A, 1) as 6 bit-planes packed one byte per 6 dst columns:
  bit pl of byte[s, j] = (A[s, 212*pl + j] >= 1). That cuts the A
  stream from 12.8MB (dense fp8) to 2.1MB per core.

  On device the DVE expands each plane with a single bitwise-AND
  (mask 0x0101<<pl on a uint16 view, two fp8 lanes per op). The masked
  bytes ARE valid fp8e4m3: a lone bit pl has the exact power-of-two
  value v_pl in [2^-9 .. 2^-3], so matmul accumulates v_pl * partial
  and the PSUM drain rescales by 2^(9-pl)... (exact powers of two).

  Cells with count >= 2 (~60 per core) are patched by one extra
  contraction chunk: their x rows are fetched on-device with an
  indirect (indexed) DMA into a 128-row tile, and a host-built
  residual matrix rmat [128, 6*212] fp8 (= (count-1) * v_pl at the
  patched columns) joins the same PSUM accumulation.

  dst is sharded across the 8 cores (core c owns cols [c*1264, ..+1264));
  no collectives. x rides as fp16 (rel err ~2e-4); out written fp16.

Per-core traffic: A-packed 2.1MB + xt 2.6MB + out 0.32MB + patch ~0.2MB
~= 5.3MB (vs 15.7MB for the dense-fp8 kernel). PE-bound at ~43us of
fp8 matmul columns.
"""

import sys

for _p in ("/opt/trn_rl_repo",):
    if _p not in sys.path:
        sys.path.append(_p)


def _install_axon_ntff_hook_shim():
    # Some images ship an antenv without axon_hooks; bass_utils then
    # crashes on trace=True under axon. Provide the module and register
    # the ctypes NTFF hook the same way trn_boot would. Fully guarded —
    # a no-op wherever the real module exists.
    import types

    try:
        import antenv.axon_hooks  # noqa: F401

        return
    except ImportError:
        pass
    try:
        import antenv

        mod = types.ModuleType("antenv.axon_hooks")
        mod._hook = None

        def set_axon_ntff_profile_hook(h):
            mod._hook = h

        def get_axon_ntff_profile_hook():
            return mod._hook

        mod.set_axon_ntff_profile_hook = set_axon_ntff_profile_hook
        mod.get_axon_ntff_profile_hook = get_axon_ntff_profile_hook
        sys.modules["antenv.axon_hooks"] = mod
        antenv.axon_hooks = mod
        from trn_agent_boot.trn_boot import _ntff_profile_via_ctypes

        mod._hook = _ntff_profile_via_ctypes("/opt/axon/libaxon_pjrt.so")
    except Exception:
        pass


_install_axon_ntff_hook_shim()

import ml_dtypes
import numpy as np

import concourse.bacc as bacc
import concourse.mybir as mybir
import concourse.tile as tile
from concourse import bass
from concourse.bass_utils import run_bass_kernel_spmd

N_NODES = 10000
D_FEAT = 128
N_CORES = 8
P = 128
KCH = -(-N_NODES // P)  # 79 source chunks
NPAD = KCH * P  # 10112
DCORE = NPAD // N_CORES  # 1264 dst columns per core
PLANES = 6
W = 212  # cols per plane; 6*212 = 1272 >= 1264
WU = W // 2  # uint16 lanes per plane
# fp8e4m3 value of a lone bit pl (exact powers of two)
BITVAL = [2.0**-9, 2.0**-8, 2.0**-7, 2.0**-6, 2.0**-5, 2.0**-3]
DRAIN_SCALE = [512.0, 256.0, 128.0, 64.0, 32.0, 8.0]
FP8 = ml_dtypes.float8_e4m3
GN = 8  # chunks per load/unpack group

# test/profiling hooks
TRACE = False
TRACE_CORES = None
LAST_RESULT = None


def _groups(sizes):
    out = []
    k0 = 0
    for g in sizes:
        out.append((k0, g))
        k0 += g
    assert k0 == KCH
    return out


KGROUPS = _groups([GN] * 9 + [KCH - 9 * GN])


def _build_program():
    nc = bacc.Bacc(
        "TRN2", target_bir_lowering=False, debug=False, num_devices=N_CORES
    )
    xt_d = nc.dram_tensor(
        "xt", [P, KCH * D_FEAT], mybir.dt.float16, kind="ExternalInput"
    )
    apk_d = nc.dram_tensor(
        "apk", [P, KCH * W], mybir.dt.uint8, kind="ExternalInput"
    )
    xr_d = nc.dram_tensor(
        "xr", [NPAD, D_FEAT], mybir.dt.float16, kind="ExternalInput"
    )
    pidx_d = nc.dram_tensor("pidx", [P, 1], mybir.dt.int32, kind="ExternalInput")
    rmat_d = nc.dram_tensor(
        "rmat", [P, PLANES * W], mybir.dt.float8e4, kind="ExternalInput"
    )
    o_d = nc.dram_tensor("o", [P, DCORE], mybir.dt.float16, kind="ExternalOutput")

    xv = xt_d[:].rearrange("p (k f) -> p k f", k=KCH, f=D_FEAT)
    av = apk_d[:].rearrange("p (k w) -> p k w", k=KCH, w=W)

    with tile.TileContext(nc) as tc:
        with (
            tc.tile_pool(name="xt", bufs=1) as xtp,
            tc.tile_pool(name="a", bufs=3) as ap_,
            tc.tile_pool(name="pl", bufs=3) as plp,
            tc.tile_pool(name="patch", bufs=1) as pp,
            tc.tile_pool(name="res", bufs=2) as resp,
            tc.tile_pool(name="ps", bufs=1, space="PSUM") as psp,
        ):
            # patch inputs ride the (otherwise idle) gpsimd queue
            pidx_sb = pp.tile([P, 1], mybir.dt.int32, name="pidx_sb")
            nc.gpsimd.dma_start(out=pidx_sb[:], in_=pidx_d[:])
            rmat_sb = pp.tile(
                [P, PLANES, W], mybir.dt.float8e4, name="rmat_sb"
            )
            nc.gpsimd.dma_start(
                out=rmat_sb[:],
                in_=rmat_d[:].rearrange("p (l w) -> p l w", l=PLANES, w=W),
            )
            xp_sb = pp.tile([P, D_FEAT], mybir.dt.float16, name="xp_sb")
            nc.gpsimd.indirect_dma_start(
                out=xp_sb[:],
                out_offset=None,
                in_=xr_d[:],
                in_offset=bass.IndirectOffsetOnAxis(ap=pidx_sb[:, :1], axis=0),
            )

            xt_sb = xtp.tile(
                [P, KCH, D_FEAT], mybir.dt.float16, tag="xt", name="xt_sb"
            )
            pss = [
                psp.tile([P, 2 * W], mybir.dt.float32, tag=f"ps{t}", name=f"ps{t}")
                for t in range(3)
            ]

            for gi, (k0, gn) in enumerate(KGROUPS):
                # x chunks on the scalar HWDGE queue, A bytes on sync —
                # both stream group-by-group so the PE starts early.
                nc.scalar.dma_start(
                    out=xt_sb[:, k0 : k0 + gn, :], in_=xv[:, k0 : k0 + gn, :]
                )
                a_sb = ap_.tile(
                    [P, gn, W], mybir.dt.uint8, tag=f"a{gn}", name=f"a{gi}"
                )
                nc.sync.dma_start(out=a_sb[:], in_=av[:, k0 : k0 + gn, :])
                pl_sb = plp.tile(
                    [P, gn, PLANES, WU], mybir.dt.uint16,
                    tag=f"pl{gn}", name=f"pl{gi}",
                )
                a_u16 = a_sb[:].bitcast(mybir.dt.uint16)  # [P, gn, WU]
                for pl in range(PLANES):
                    nc.vector.tensor_scalar(
                        out=pl_sb[:, :, pl, :],
                        in0=a_u16,
                        scalar1=(0x0101 << pl) & 0xFFFF,
                        scalar2=None,
                        op0=mybir.AluOpType.bitwise_and,
                    )
                for kk in range(gn):
                    for t in range(3):
                        rhs = pl_sb[:, kk, 2 * t : 2 * t + 2, :].bitcast(
                            mybir.dt.float8e4
                        )  # [P, 2, W] fp8 = 424 cols
                        nc.tensor.matmul(
                            pss[t][:],
                            xt_sb[:, k0 + kk, :],
                            rhs,
                            start=(k0 + kk == 0),
                            stop=False,
                        )
            # patch chunk: scattered count>=2 residuals
            for t in range(3):
                nc.tensor.matmul(
                    pss[t][:],
                    xp_sb[:],
                    rmat_sb[:, 2 * t : 2 * t + 2, :],
                    start=False,
                    stop=True,
                )
            # drain: rescale each plane (exact powers of 2) to fp16 out
            for t in range(3):
                w_hi = W if t < 2 else DCORE - (2 * t + 1) * W  # 212/212/204
                res = resp.tile(
                    [P, W + w_hi], mybir.dt.float16, tag=f"res{t}", name=f"res{t}"
                )
                nc.vector.tensor_scalar(
                    out=res[:, :W],
                    in0=pss[t][:, :W],
                    scalar1=DRAIN_SCALE[2 * t],
                    scalar2=None,
                    op0=mybir.AluOpType.mult,
                )
                nc.vector.tensor_scalar(
                    out=res[:, W : W + w_hi],
                    in0=pss[t][:, W : W + w_hi],
                    scalar1=DRAIN_SCALE[2 * t + 1],
                    scalar2=None,
                    op0=mybir.AluOpType.mult,
                )
                eng = nc.sync if t == 0 else nc.scalar
                eng.dma_start(
                    out=o_d[:, 2 * W * t : 2 * W * t + W + w_hi], in_=res[:]
                )

    nc.compile()
    return nc


def _prepare(x: np.ndarray, edge_index: np.ndarray):
    ei = np.asarray(edge_index)
    src = ei[0].astype(np.int64)
    dst = ei[1].astype(np.int64)

    xf = np.asarray(x).astype(np.float32)
    xp = np.zeros((NPAD, D_FEAT), np.float16)
    xp[:N_NODES] = xf
    # xt[p, k, :] = x[k*128 + p, :]
    xt = np.ascontiguousarray(
        xp.reshape(KCH, P, D_FEAT).transpose(1, 0, 2).reshape(P, KCH * D_FEAT)
    )

    shifts = (1 << np.arange(PLANES, dtype=np.uint32))[None, :, None]
    bitvals = np.array(BITVAL, np.float32)

    in_maps = []
    for c in range(N_CORES):
        sel = (dst >= c * DCORE) & (dst < (c + 1) * DCORE)
        idx = src[sel] * DCORE + (dst[sel] - c * DCORE)
        cnt = np.bincount(idx, minlength=NPAD * DCORE).reshape(NPAD, DCORE)
        base = np.minimum(cnt, 1)

        g = np.zeros((NPAD, PLANES, W), np.uint32)
        g.reshape(NPAD, PLANES * W)[:, :DCORE] = base
        byte = (g * shifts).sum(axis=1).astype(np.uint8)  # [NPAD, W]
        apk = np.ascontiguousarray(
            byte.reshape(KCH, P, W).transpose(1, 0, 2).reshape(P, KCH * W)
        )

        resid = (cnt - base).astype(np.int64)
        rs, cs = np.nonzero(resid)
        uniq = np.unique(rs)
        assert len(uniq) <= P, f"core {c}: {len(uniq)} patch rows > {P}"
        assert resid.max(initial=0) <= 15
        slot_of = np.zeros(NPAD, np.int64)
        slot_of[uniq] = np.arange(len(uniq))
        pidx = np.zeros((P, 1), np.int32)
        pidx[: len(uniq), 0] = uniq.astype(np.int32)
        rmat = np.zeros((P, PLANES * W), np.float32)
        if len(rs):
            pl = cs // W
            j = cs % W
            rmat[slot_of[rs], pl * W + j] = resid[rs, cs] * bitvals[pl]
        in_maps.append(
            {
                "xt": xt,
                "apk": apk,
                "xr": xp,
                "pidx": pidx,
                "rmat": rmat.astype(FP8),
            }
        )
    return in_maps


def kernel(x: np.ndarray, edge_index: np.ndarray) -> np.ndarray:
    global LAST_RESULT
    in_maps = _prepare(x, edge_index)
    nc = _build_program()
    res = run_bass_kernel_spmd(
        nc,
        in_maps,
        list(range(N_CORES)),
        trace=TRACE,
        trace_cores=TRACE_CORES if TRACE else None,
    )
    LAST_RESULT = res
    # o per core: [128 f, DCORE d] fp16 -> out[c*DCORE + d, f] f32
    out = np.concatenate(
        [np.asarray(r["o"]).astype(np.float32).T for r in res.results], axis=0
    )
    return np.ascontiguousarray(out[:N_NODES])


if __name__ == "__main__":
    rng = np.random.default_rng(0)
    x = rng.standard_normal((N_NODES, D_FEAT), dtype=np.float32)
    edge_index = rng.integers(0, N_NODES, size=(2, 320000)).astype(np.int64)
    out = kernel(x, edge_index)
    ref = np.zeros((N_NODES, D_FEAT), np.float32)
    np.add.at(ref, edge_index[1], x[edge_index[0]])
    rel = np.linalg.norm(out - ref) / np.linalg.norm(ref)
    print("rel L2 err:", rel)
